# revision 18
# baseline (speedup 1.0000x reference)
"""Multi-head causal attention with RoPE for TRN2, sharded over 8 NeuronCores. v4.

Sharding: 2-way data parallel over batch x 4-way tensor parallel over heads.
Core c handles batch c//4 and heads [4*(c%4), 4*(c%4)+4).

v4 changes vs v2 (201us) / v3 (238us):
  - Interleaved schedule with normalize-hiding: PE order is
    [P0, A0, P1, Z0 O0, A1, P2, Z1 O1, A2, P3, Z2 O2, A3(+Z3a), Z3b O3] so
    the serial normalize chain of row sc hides behind projections of sc+1,
    and row 3 normalizes pair 0 between heads 2 and 3.
  - Host pre-arranges weights partition-major ([128, 8, 256] etc.) so weight
    DMAs are 2-4KB-contiguous per partition instead of 512B-descriptor
    rearranges (the v2/v3 wq rearrange DMA alone took 6.7us and gated the
    first matmul).
  - DMA queues: scalar=weights only; sync=x evens + wv + po; gpsimd=x odds +
    tables + zq gathers.  No DMA issue ever sits ahead of latency-critical
    engine work on the same queue (v3's regression).
  - Rope entirely on DVE; ACT does only exp; po evict on DVE; bf16 po.
  - Per-head-pair normalization (sel2 [2,256] selector).
"""
import sys

sys.path.insert(0, "/opt/trn_rl_repo")

import numpy as np
import ml_dtypes
import concourse.bass as bass
import concourse.mybir as mybir
import concourse.tile as tile
from concourse import bacc
from concourse.bass_utils import run_bass_kernel_spmd

D = 1024          # d_model
H = 16            # total heads
DH = 64           # head dim
S = 2048          # sequence length
B = 2             # batch
NCORES = 8
HPC = 4           # heads per core
DHC = HPC * DH    # head dims per core = 256
ROPE_THETA = 10000.0

F32 = mybir.dt.float32
F32R = mybir.dt.float32r
BF16 = mybir.dt.bfloat16

SC = 512          # seq chunk for matmul N dim
NSC = S // SC     # 4
NJT = D // 128    # 8 contraction tiles
NST = S // 128    # 16 s-tiles

BF16NP = ml_dtypes.bfloat16

# rope row permutation within one head (64 rows):
PERM64 = ([2 * i for i in range(16)] + [2 * i + 1 for i in range(16)]
          + [2 * i for i in range(16, 32)] + [2 * i + 1 for i in range(16, 32)])
SHUF_MASK = [(r + 16) % 32 for r in range(32)]  # a<->b swap within each 32-quadrant


def _rope_tables():
    """cos/sin tables [128, S] for the permuted 2-head row layout."""
    inv = ROPE_THETA ** (-np.arange(32, dtype=np.float64) * 2.0 / 64.0)
    pos = np.arange(S, dtype=np.float64)
    r = np.arange(128)
    q, rr = r // 32, r % 32
    f = (q % 2) * 16 + (rr % 16)
    sign = np.where(rr < 16, -1.0, 1.0)
    ang = pos[None, :] * inv[f][:, None]            # [128, S]
    ct = np.cos(ang)
    st = sign[:, None] * np.sin(ang)
    return ct.astype(BF16NP), st.astype(BF16NP)


def build(repeat: int = 1):
    nc = bacc.Bacc(None, target_bir_lowering=False)

    xT = nc.dram_tensor("xT", [D, S], BF16, kind="ExternalInput")
    wq = nc.dram_tensor("wq", [128, NJT, DHC], BF16, kind="ExternalInput")
    wk = nc.dram_tensor("wk", [128, NJT, DHC], BF16, kind="ExternalInput")
    wv = nc.dram_tensor("wv", [128, NJT, DHC], BF16, kind="ExternalInput")
    wo = nc.dram_tensor("wo", [128, 2, D], BF16, kind="ExternalInput")
    ct = nc.dram_tensor("ct", [128, S], BF16, kind="ExternalInput")
    st = nc.dram_tensor("st", [128, S], BF16, kind="ExternalInput")
    msk = nc.dram_tensor("msk", [128, 128], BF16, kind="ExternalInput")
    sel = nc.dram_tensor("sel", [2, 256], BF16, kind="ExternalInput")
    po = nc.dram_tensor("po", [D, S], BF16, kind="ExternalOutput")

    with tile.TileContext(nc) as tc:
        def body(_iv=None):
            _build_body(nc, tc, xT, wq, wk, wv, wo, ct, st, msk, sel, po)

        if repeat == 1:
            body()
        else:
            with tc.For_i(0, repeat, 1) as iv:
                body(iv)

    nc.compile()
    return nc


def _build_body(nc, tc, xT, wq, wk, wv, wo, ct, st, msk, sel, po):
    from contextlib import ExitStack
    with ExitStack() as ctx:
        # ---- persistent tiles ----
        pers = ctx.enter_context(tc.tile_pool(name="pers", bufs=1))
        qk_sb = [pers.tile([128, S], BF16, tag=f"qk{i}", name=f"qk{i}") for i in range(4)]
        # qk_sb[0..1] = q head-pairs 0,1; [2..3] = k head-pairs 0,1
        v_sb = [pers.tile([128, HPC * 65], BF16, tag=f"v{i}", name=f"v{i}")
                for i in range(NST)]
        ho_sb = [pers.tile([128, S], BF16, tag=f"ho{i}", name=f"ho{i}") for i in range(2)]
        xt = [pers.tile([128, S], BF16, tag=f"x{j}", name=f"x{j}") for j in range(NJT)]
        wqs = pers.tile([128, NJT, DHC], BF16, tag="wqs")
        wks = pers.tile([128, NJT, DHC], BF16, tag="wks")
        wvs = pers.tile([128, NJT, DHC], BF16, tag="wvs")
        wos = pers.tile([128, 2, D], BF16, tag="wos")
        cts = pers.tile([128, S], BF16, tag="cts")
        sts = pers.tile([128, S], BF16, tag="sts")
        msks = pers.tile([128, 128], BF16, tag="msks")
        sels = pers.tile([2, 256], BF16, tag="sels")

        # ---- DMA staging ----
        # scalar: weights only (first exp isn't needed until ~15us).
        nc.scalar.dma_start(out=wqs[:, 0:4, :], in_=wq[:, 0:4, :])
        nc.scalar.dma_start(out=wqs[:, 4:8, :], in_=wq[:, 4:8, :])
        nc.scalar.dma_start(out=wks[:, 0:4, :], in_=wk[:, 0:4, :])
        nc.scalar.dma_start(out=wks[:, 4:8, :], in_=wk[:, 4:8, :])
        nc.scalar.dma_start(out=wos, in_=wo[:, :, :])
        # sync: x chunk-0 even j, wv, sel; later the po writes.
        for j in range(0, NJT, 2):
            nc.sync.dma_start(out=xt[j][:, 0:SC], in_=xT[j * 128:(j + 1) * 128, 0:SC])
        nc.sync.dma_start(out=wvs[:, 0:4, :], in_=wv[:, 0:4, :])
        nc.sync.dma_start(out=wvs[:, 4:8, :], in_=wv[:, 4:8, :])
        nc.sync.dma_start(out=sels, in_=sel[:, :])
        for j in range(0, NJT, 2):
            nc.sync.dma_start(out=xt[j][:, SC:S], in_=xT[j * 128:(j + 1) * 128, SC:S])
        # gpsimd: x chunk-0 odd j, mask, rope tables; later the zq gathers.
        for j in range(1, NJT, 2):
            nc.gpsimd.dma_start(out=xt[j][:, 0:SC], in_=xT[j * 128:(j + 1) * 128, 0:SC])
        nc.gpsimd.dma_start(out=msks, in_=msk[:, :])
        nc.gpsimd.dma_start(out=cts[:, 0:SC], in_=ct[:, 0:SC])
        nc.gpsimd.dma_start(out=sts[:, 0:SC], in_=st[:, 0:SC])
        for j in range(1, NJT, 2):
            nc.gpsimd.dma_start(out=xt[j][:, SC:S], in_=xT[j * 128:(j + 1) * 128, SC:S])
        nc.gpsimd.dma_start(out=cts[:, SC:S], in_=ct[:, SC:S])
        nc.gpsimd.dma_start(out=sts[:, SC:S], in_=st[:, SC:S])

        with tc.tile_pool(name="ps", bufs=2, space="PSUM") as ps_pool, \
             tc.tile_pool(name="rtmp", bufs=3) as rtmp, \
             tc.tile_pool(name="att", bufs=5) as att, \
             tc.tile_pool(name="hozp", bufs=6) as hozp, \
             tc.tile_pool(name="zp", bufs=2) as zp, \
             tc.tile_pool(name="pop", bufs=3) as pop:

            def proj(sc):
                csl = cts[:, sc * SC:(sc + 1) * SC]
                ssl = sts[:, sc * SC:(sc + 1) * SC]
                for qi, ws in ((0, wqs), (1, wks)):
                    for t in range(2):          # head-pair tile
                        dst = qk_sb[qi * 2 + t]
                        ps = ps_pool.tile([128, SC], F32, tag="mm", name="ps")
                        for j in range(NJT):
                            nc.tensor.matmul(
                                ps[:, :],
                                ws[:, j, t * 128:(t + 1) * 128],
                                xt[j][:, sc * SC:(sc + 1) * SC],
                                start=(j == 0), stop=(j == NJT - 1))
                        ev = rtmp.tile([128, SC], BF16, tag="ev", name="ev")
                        nc.vector.tensor_copy(ev, ps[:, :])
                        sp = rtmp.tile([128, SC], BF16, tag="sp", name="sp")
                        nc.vector.stream_shuffle(sp, ev, SHUF_MASK)
                        t1 = rtmp.tile([128, SC], BF16, tag="t1", name="t1")
                        t2 = rtmp.tile([128, SC], BF16, tag="t2", name="t2")
                        nc.vector.tensor_mul(t1, ev, csl)
                        nc.vector.tensor_mul(t2, sp, ssl)
                        nc.vector.tensor_add(
                            dst[:, sc * SC:(sc + 1) * SC], t1, t2)
                for si in range(sc * 4, sc * 4 + 4):
                    ps = ps_pool.tile([128, DHC], F32, tag="mm", name="psv")
                    for j in range(NJT):
                        nc.tensor.matmul(
                            ps[:, :],
                            xt[j][:, si * 128:(si + 1) * 128],
                            wvs[:, j, :],
                            start=(j == 0), stop=(j == NJT - 1))
                    nc.vector.tensor_copy(
                        v_sb[si].rearrange("p (h e) -> p h e", e=65)[:, :, 0:64],
                        ps.rearrange("p (h e) -> p h e", e=64)[:, :, :])
                    nc.gpsimd.memset(
                        v_sb[si].rearrange("p (h e) -> p h e", e=65)[:, :, 64:65],
                        1.0)

            def att_head(qc, h, zqt, zrow=0, zq_sync=False):
                t, hh = h // 2, h % 2
                q_t = qk_sb[t]
                k_t = qk_sb[2 + t]
                rows = slice(hh * 64, hh * 64 + 64)
                nkt = 4 * qc + 4
                pso = ps_pool.tile([65, SC], F32, tag="pso", name="pso")
                for kb in range(0, nkt, 2):
                    qkps = ps_pool.tile([128, 2 * SC], F32, tag="qkps", name="qkps")
                    offs = []
                    for half, kt in enumerate((kb, kb + 1)):
                        off = max(0, kt * 128 - qc * SC)
                        offs.append(off)
                        nc.tensor.matmul(
                            qkps[:, half * SC + off:(half + 1) * SC],
                            k_t[rows, kt * 128:(kt + 1) * 128],
                            q_t[rows, qc * SC + off:(qc + 1) * SC],
                            start=True, stop=True)
                    ex = att.tile([128, 2 * SC], BF16, tag="ex", name="ex")
                    if offs[0] == 0 and offs[1] == 0:
                        nc.scalar.activation(ex[:, :], qkps[:, :],
                                             mybir.ActivationFunctionType.Exp)
                    else:
                        for half in range(2):
                            o = half * SC + offs[half]
                            nc.scalar.activation(
                                ex[:, o:(half + 1) * SC],
                                qkps[:, o:(half + 1) * SC],
                                mybir.ActivationFunctionType.Exp)
                    for half, kt in enumerate((kb, kb + 1)):
                        off = offs[half]
                        if kt * 128 >= qc * SC:  # diagonal: causal 0/1 mask
                            o = half * SC + off
                            nc.vector.tensor_mul(
                                ex[:, o:o + 128], ex[:, o:o + 128], msks)
                        nc.tensor.matmul(
                            pso[:, off:],
                            v_sb[kt][:, h * 65:h * 65 + 65],
                            ex[:, half * SC + off:(half + 1) * SC],
                            start=(kt == 0), stop=(kt == nkt - 1))
                # evict unnormalized out + Z row; free the pso bank
                hz = hozp.tile([65, SC], F32, tag="hz", name="hz")
                nc.vector.tensor_copy(hz, pso[:, :])
                eng = nc.sync if zq_sync else nc.gpsimd
                eng.dma_start(out=zqt[zrow:zrow + 1, :], in_=hz[64:65, :])
                return hz

            def zb_pair(qc, t, hoz_pair, zqt):
                """normalize head pair t of row qc (bc broadcast + muls)."""
                rz = zp.tile([2, SC], F32, tag="rz", name="rz")
                nc.vector.reciprocal_approx_fast(rz, zqt[:, :])
                rzr = zp.tile([2, SC], BF16, tag="rzr", name="rzr")
                nc.vector.tensor_copy(rzr, rz)
                bc = ps_pool.tile([128, SC], F32, tag="mm", name="bc")
                nc.tensor.matmul(
                    bc[:, :], sels[:, t * 128:(t + 1) * 128],
                    rzr[:, :], start=True, stop=True)
                for hh in range(2):
                    rows = slice(hh * 64, hh * 64 + 64)
                    nc.vector.tensor_mul(
                        ho_sb[t][rows, qc * SC:(qc + 1) * SC],
                        hoz_pair[hh][0:64, :], bc[rows, :])

            def zb_head(qc, h, hz, zqt):
                """single-head normalize (short critical path for last row)."""
                t, hh = h // 2, h % 2
                rz = zp.tile([1, SC], F32, tag="rz1", name="rz1")
                nc.vector.reciprocal_approx_fast(rz, zqt[0:1, :])
                rzr = zp.tile([1, SC], BF16, tag="rzr1", name="rzr1")
                nc.vector.tensor_copy(rzr, rz)
                bc = ps_pool.tile([64, SC], F32, tag="pso", name="bch")
                # sels[0, 0:64] is all-ones: broadcast rzr across 64 rows
                nc.tensor.matmul(
                    bc[:, :], sels[0:1, 0:64],
                    rzr[:, :], start=True, stop=True)
                nc.vector.tensor_mul(
                    ho_sb[t][hh * 64:(hh + 1) * 64, qc * SC:(qc + 1) * SC],
                    hz[0:64, :], bc[:, :])

            def oproj(qc, split_evict=False):
                for mt in range(NJT):
                    tag = "mm" if mt % 2 == 0 else "pso"
                    ps = ps_pool.tile([128, SC], F32, tag=tag, name="pso2")
                    for it in range(2):
                        nc.tensor.matmul(
                            ps[:, :],
                            wos[:, it, mt * 128:(mt + 1) * 128],
                            ho_sb[it][:, qc * SC:(qc + 1) * SC],
                            start=(it == 0), stop=(it == 1))
                    pe = pop.tile([128, SC], BF16, tag="pe", name="pe")
                    if split_evict and mt % 2 == 1:
                        nc.scalar.copy(pe, ps[:, :])
                        dma_eng = nc.scalar
                    else:
                        nc.vector.tensor_copy(pe, ps[:, :])
                        dma_eng = nc.sync
                    dma_eng.dma_start(
                        out=po[mt * 128:(mt + 1) * 128, qc * SC:(qc + 1) * SC],
                        in_=pe)

            # ---- schedule ----
            proj(0)
            for sc in range(NSC - 1):
                zq0 = zp.tile([2, SC], F32, tag="zq", name="zq0")
                zq1 = zp.tile([2, SC], F32, tag="zq", name="zq1")
                hoz = [att_head(sc, h, (zq0, zq1)[h // 2], zrow=h % 2)
                       for h in range(HPC)]
                proj(sc + 1)
                zb_pair(sc, 0, hoz[0:2], zq0)
                zb_pair(sc, 1, hoz[2:4], zq1)
                oproj(sc)
            # last row: per-head normalize for heads 2/3 so o-proj starts asap
            sc = NSC - 1
            zq0 = zp.tile([2, SC], F32, tag="zq", name="zq0")
            zq2 = zp.tile([1, SC], F32, tag="zq1", name="zq2")
            zq3 = zp.tile([1, SC], F32, tag="zq1", name="zq3")
            hoz = [att_head(sc, h, (zq0, zq0, zq2)[h], zrow=h % 2 if h < 2 else 0)
                   for h in range(3)]
            zb_pair(sc, 0, hoz[0:2], zq0)
            hoz.append(att_head(sc, 3, zq3, zrow=0, zq_sync=True))
            zb_head(sc, 2, hoz[2], zq2)
            zb_head(sc, 3, hoz[3], zq3)
            oproj(sc, split_evict=True)


_NC_CACHE = {}


def _get_nc(repeat: int = 1):
    if repeat not in _NC_CACHE:
        _NC_CACHE[repeat] = build(repeat)
    return _NC_CACHE[repeat]


def _host_prep(q_weight, k_weight, v_weight, o_weight, in_features):
    """Build the 8 per-core input maps."""
    ct, st = _rope_tables()
    # mask in [k, q] layout: allow k <= q
    mask01 = (np.arange(128)[:, None] <= np.arange(128)[None, :]).astype(BF16NP)
    # sel2[i, t*128+m] = 1 where i = m//64 (within-pair broadcast selector)
    sel = np.zeros((2, 256), dtype=np.float32)
    for t in range(2):
        for m in range(128):
            sel[m // 64, t * 128 + m] = 1.0
    sel = sel.astype(BF16NP)

    qw = q_weight.reshape(H, DH, D)
    kw = k_weight.reshape(H, DH, D)
    vw = v_weight.reshape(H, DH, D)

    def part_major(w_dm):  # [D, M] -> [128, NJT, M] partition-major
        return np.ascontiguousarray(
            w_dm.reshape(NJT, 128, w_dm.shape[1]).transpose(1, 0, 2))

    in_maps = []
    for c in range(NCORES):
        b, g = c // 4, c % 4
        heads = list(range(4 * g, 4 * g + 4))
        wq_c = part_major(np.ascontiguousarray(
            (0.125 * qw[heads][:, PERM64, :]).reshape(DHC, D).T).astype(BF16NP))
        wk_c = part_major(np.ascontiguousarray(
            kw[heads][:, PERM64, :].reshape(DHC, D).T).astype(BF16NP))
        wv_c = part_major(np.ascontiguousarray(
            vw[heads].reshape(DHC, D).T).astype(BF16NP))
        wo_flat = np.ascontiguousarray(
            o_weight[:, 4 * g * DH:(4 * g + 4) * DH].T).astype(BF16NP)  # [DHC, D]
        wo_c = np.ascontiguousarray(
            wo_flat.reshape(2, 128, D).transpose(1, 0, 2))
        xT_c = np.ascontiguousarray(in_features[b].T).astype(BF16NP)
        in_maps.append({
            "xT": xT_c, "wq": wq_c, "wk": wk_c, "wv": wv_c, "wo": wo_c,
            "ct": ct, "st": st, "msk": mask01,
            "sel": sel,
        })
    return in_maps


def kernel(q_weight, k_weight, v_weight, o_weight, in_features):
    q_weight = np.asarray(q_weight, dtype=np.float32)
    k_weight = np.asarray(k_weight, dtype=np.float32)
    v_weight = np.asarray(v_weight, dtype=np.float32)
    o_weight = np.asarray(o_weight, dtype=np.float32)
    in_features = np.asarray(in_features, dtype=np.float32)

    nc = _get_nc(1)
    in_maps = _host_prep(q_weight, k_weight, v_weight, o_weight, in_features)
    res = run_bass_kernel_spmd(nc, in_maps, core_ids=list(range(NCORES)))

    out = np.empty((B, S, D), dtype=np.float32)
    for b in range(B):
        acc = res.results[4 * b]["po"].astype(np.float32)
        for g in range(1, 4):
            acc += res.results[4 * b + g]["po"].astype(np.float32)
        out[b] = acc.T
    return out


# revision 22
# speedup vs baseline: 1.0572x; 1.0572x over previous
"""Multi-head causal attention with RoPE for TRN2, sharded over 8 NeuronCores. v4.

Sharding: 2-way data parallel over batch x 4-way tensor parallel over heads.
Core c handles batch c//4 and heads [4*(c%4), 4*(c%4)+4).

v4 changes vs v2 (201us) / v3 (238us):
  - Interleaved schedule with normalize-hiding: PE order is
    [P0, A0, P1, Z0 O0, A1, P2, Z1 O1, A2, P3, Z2 O2, A3(+Z3a), Z3b O3] so
    the serial normalize chain of row sc hides behind projections of sc+1,
    and row 3 normalizes pair 0 between heads 2 and 3.
  - Host pre-arranges weights partition-major ([128, 8, 256] etc.) so weight
    DMAs are 2-4KB-contiguous per partition instead of 512B-descriptor
    rearranges (the v2/v3 wq rearrange DMA alone took 6.7us and gated the
    first matmul).
  - DMA queues: scalar=weights only; sync=x evens + wv + po; gpsimd=x odds +
    tables + zq gathers.  No DMA issue ever sits ahead of latency-critical
    engine work on the same queue (v3's regression).
  - Rope entirely on DVE; ACT does only exp; po evict on DVE; bf16 po.
  - Per-head-pair normalization (sel2 [2,256] selector).
"""
import sys

sys.path.insert(0, "/opt/trn_rl_repo")

import numpy as np
import ml_dtypes
import concourse.bass as bass
import concourse.mybir as mybir
import concourse.tile as tile
from concourse import bacc
from concourse.bass_utils import run_bass_kernel_spmd

D = 1024          # d_model
H = 16            # total heads
DH = 64           # head dim
S = 2048          # sequence length
B = 2             # batch
NCORES = 8
HPC = 4           # heads per core
DHC = HPC * DH    # head dims per core = 256
ROPE_THETA = 10000.0

F32 = mybir.dt.float32
F32R = mybir.dt.float32r
BF16 = mybir.dt.bfloat16

SC = 512          # seq chunk for matmul N dim
NSC = S // SC     # 4
NJT = D // 128    # 8 contraction tiles
NST = S // 128    # 16 s-tiles

BF16NP = ml_dtypes.bfloat16

# rope row permutation within one head (64 rows):
PERM64 = ([2 * i for i in range(16)] + [2 * i + 1 for i in range(16)]
          + [2 * i for i in range(16, 32)] + [2 * i + 1 for i in range(16, 32)])
SHUF_MASK = [(r + 16) % 32 for r in range(32)]  # a<->b swap within each 32-quadrant


def _rope_tables():
    """cos/sin tables [128, S] for the permuted 2-head row layout."""
    inv = ROPE_THETA ** (-np.arange(32, dtype=np.float64) * 2.0 / 64.0)
    pos = np.arange(S, dtype=np.float64)
    r = np.arange(128)
    q, rr = r // 32, r % 32
    f = (q % 2) * 16 + (rr % 16)
    sign = np.where(rr < 16, -1.0, 1.0)
    ang = pos[None, :] * inv[f][:, None]            # [128, S]
    ct = np.cos(ang)
    st = sign[:, None] * np.sin(ang)
    return ct.astype(BF16NP), st.astype(BF16NP)


def build(repeat: int = 1):
    nc = bacc.Bacc(None, target_bir_lowering=False)

    xT = nc.dram_tensor("xT", [D, S], BF16, kind="ExternalInput")
    wq = nc.dram_tensor("wq", [128, NJT, DHC], BF16, kind="ExternalInput")
    wk = nc.dram_tensor("wk", [128, NJT, DHC], BF16, kind="ExternalInput")
    wv = nc.dram_tensor("wv", [128, NJT, DHC], BF16, kind="ExternalInput")
    wo = nc.dram_tensor("wo", [128, 2, D], BF16, kind="ExternalInput")
    ct = nc.dram_tensor("ct", [128, S], BF16, kind="ExternalInput")
    st = nc.dram_tensor("st", [128, S], BF16, kind="ExternalInput")
    msk = nc.dram_tensor("msk", [128, 128], BF16, kind="ExternalInput")
    sel = nc.dram_tensor("sel", [2, 256], BF16, kind="ExternalInput")
    po = nc.dram_tensor("po", [D, S], BF16, kind="ExternalOutput")

    with tile.TileContext(nc) as tc:
        def body(_iv=None):
            _build_body(nc, tc, xT, wq, wk, wv, wo, ct, st, msk, sel, po)

        if repeat == 1:
            body()
        else:
            with tc.For_i(0, repeat, 1) as iv:
                body(iv)

    nc.compile()
    return nc


def _build_body(nc, tc, xT, wq, wk, wv, wo, ct, st, msk, sel, po):
    from contextlib import ExitStack
    with ExitStack() as ctx:
        # ---- persistent tiles ----
        pers = ctx.enter_context(tc.tile_pool(name="pers", bufs=1))
        qk_sb = [pers.tile([128, S], BF16, tag=f"qk{i}", name=f"qk{i}") for i in range(4)]
        # qk_sb[0..1] = q head-pairs 0,1; [2..3] = k head-pairs 0,1
        v_sb = [pers.tile([128, HPC * 65], BF16, tag=f"v{i}", name=f"v{i}")
                for i in range(NST)]
        ho_sb = [pers.tile([128, S], BF16, tag=f"ho{i}", name=f"ho{i}") for i in range(2)]
        xt = [pers.tile([128, S], BF16, tag=f"x{j}", name=f"x{j}") for j in range(NJT)]
        wqs = pers.tile([128, NJT, DHC], BF16, tag="wqs")
        wks = pers.tile([128, NJT, DHC], BF16, tag="wks")
        wvs = pers.tile([128, NJT, DHC], BF16, tag="wvs")
        wos = pers.tile([128, 2, D], BF16, tag="wos")
        cts = pers.tile([128, S], BF16, tag="cts")
        sts = pers.tile([128, S], BF16, tag="sts")
        msks = pers.tile([128, 128], BF16, tag="msks")
        sels = pers.tile([2, 256], BF16, tag="sels")

        # ---- DMA staging ----
        # scalar: weights only (first exp isn't needed until ~15us).
        nc.scalar.dma_start(out=wqs[:, 0:4, :], in_=wq[:, 0:4, :])
        nc.scalar.dma_start(out=wqs[:, 4:8, :], in_=wq[:, 4:8, :])
        nc.scalar.dma_start(out=wks[:, 0:4, :], in_=wk[:, 0:4, :])
        nc.scalar.dma_start(out=wks[:, 4:8, :], in_=wk[:, 4:8, :])
        nc.scalar.dma_start(out=wos, in_=wo[:, :, :])
        # sync: x chunk-0 even j, wv, sel; later the po writes.
        for j in range(0, NJT, 2):
            nc.sync.dma_start(out=xt[j][:, 0:SC], in_=xT[j * 128:(j + 1) * 128, 0:SC])
        nc.sync.dma_start(out=wvs[:, 0:4, :], in_=wv[:, 0:4, :])
        nc.sync.dma_start(out=wvs[:, 4:8, :], in_=wv[:, 4:8, :])
        nc.sync.dma_start(out=sels, in_=sel[:, :])
        for j in range(0, NJT, 2):
            nc.sync.dma_start(out=xt[j][:, SC:S], in_=xT[j * 128:(j + 1) * 128, SC:S])
        # gpsimd: x chunk-0 odd j, mask, rope tables; later the zq gathers.
        for j in range(1, NJT, 2):
            nc.gpsimd.dma_start(out=xt[j][:, 0:SC], in_=xT[j * 128:(j + 1) * 128, 0:SC])
        nc.gpsimd.dma_start(out=msks, in_=msk[:, :])
        nc.gpsimd.dma_start(out=cts[:, 0:SC], in_=ct[:, 0:SC])
        nc.gpsimd.dma_start(out=sts[:, 0:SC], in_=st[:, 0:SC])
        for j in range(1, NJT, 2):
            nc.gpsimd.dma_start(out=xt[j][:, SC:S], in_=xT[j * 128:(j + 1) * 128, SC:S])
        nc.gpsimd.dma_start(out=cts[:, SC:S], in_=ct[:, SC:S])
        nc.gpsimd.dma_start(out=sts[:, SC:S], in_=st[:, SC:S])

        with tc.tile_pool(name="ps", bufs=2, space="PSUM") as ps_pool, \
             tc.tile_pool(name="rtmp", bufs=3) as rtmp, \
             tc.tile_pool(name="att", bufs=5) as att, \
             tc.tile_pool(name="hozp", bufs=6) as hozp, \
             tc.tile_pool(name="zp", bufs=2) as zp, \
             tc.tile_pool(name="pop", bufs=3) as pop:

            def proj(sc):
                csl = cts[:, sc * SC:(sc + 1) * SC]
                ssl = sts[:, sc * SC:(sc + 1) * SC]
                for t in range(2):              # head-pair tile
                    for qi, ws in ((0, wqs), (1, wks)):
                        dst = qk_sb[qi * 2 + t]
                        ps = ps_pool.tile([128, SC], F32, tag="mm", name="ps")
                        for j in range(NJT):
                            nc.tensor.matmul(
                                ps[:, :],
                                ws[:, j, t * 128:(t + 1) * 128],
                                xt[j][:, sc * SC:(sc + 1) * SC],
                                start=(j == 0), stop=(j == NJT - 1))
                        ev = rtmp.tile([128, SC], BF16, tag="ev", name="ev")
                        nc.vector.tensor_copy(ev, ps[:, :])
                        sp = rtmp.tile([128, SC], BF16, tag="sp", name="sp")
                        nc.vector.stream_shuffle(sp, ev, SHUF_MASK)
                        t1 = rtmp.tile([128, SC], BF16, tag="t1", name="t1")
                        t2 = rtmp.tile([128, SC], BF16, tag="t2", name="t2")
                        nc.vector.tensor_mul(t1, ev, csl)
                        nc.vector.tensor_mul(t2, sp, ssl)
                        nc.vector.tensor_add(
                            dst[:, sc * SC:(sc + 1) * SC], t1, t2)
                for si in range(sc * 4, sc * 4 + 4):
                    ps = ps_pool.tile([128, DHC], F32, tag="mm", name="psv")
                    for j in range(NJT):
                        nc.tensor.matmul(
                            ps[:, :],
                            xt[j][:, si * 128:(si + 1) * 128],
                            wvs[:, j, :],
                            start=(j == 0), stop=(j == NJT - 1))
                    nc.vector.tensor_copy(
                        v_sb[si].rearrange("p (h e) -> p h e", e=65)[:, :, 0:64],
                        ps.rearrange("p (h e) -> p h e", e=64)[:, :, :])
                    nc.gpsimd.memset(
                        v_sb[si].rearrange("p (h e) -> p h e", e=65)[:, :, 64:65],
                        1.0)

            def att_head(qc, h, zqt, zrow=0, zq_sync=False):
                t, hh = h // 2, h % 2
                q_t = qk_sb[t]
                k_t = qk_sb[2 + t]
                rows = slice(hh * 64, hh * 64 + 64)
                nkt = 4 * qc + 4
                pso = ps_pool.tile([65, SC], F32, tag="pso", name="pso")
                for kb in range(0, nkt, 2):
                    qkps = ps_pool.tile([128, 2 * SC], F32, tag="qkps", name="qkps")
                    offs = []
                    for half, kt in enumerate((kb, kb + 1)):
                        off = max(0, kt * 128 - qc * SC)
                        offs.append(off)
                        nc.tensor.matmul(
                            qkps[:, half * SC + off:(half + 1) * SC],
                            k_t[rows, kt * 128:(kt + 1) * 128],
                            q_t[rows, qc * SC + off:(qc + 1) * SC],
                            start=True, stop=True)
                    ex = att.tile([128, 2 * SC], BF16, tag="ex", name="ex")
                    if offs[0] == 0 and offs[1] == 0:
                        nc.scalar.activation(ex[:, :], qkps[:, :],
                                             mybir.ActivationFunctionType.Exp)
                    else:
                        for half in range(2):
                            o = half * SC + offs[half]
                            nc.scalar.activation(
                                ex[:, o:(half + 1) * SC],
                                qkps[:, o:(half + 1) * SC],
                                mybir.ActivationFunctionType.Exp)
                    for half, kt in enumerate((kb, kb + 1)):
                        off = offs[half]
                        if kt * 128 >= qc * SC:  # diagonal: causal 0/1 mask
                            o = half * SC + off
                            nc.vector.tensor_mul(
                                ex[:, o:o + 128], ex[:, o:o + 128], msks)
                        nc.tensor.matmul(
                            pso[:, off:],
                            v_sb[kt][:, h * 65:h * 65 + 65],
                            ex[:, half * SC + off:(half + 1) * SC],
                            start=(kt == 0), stop=(kt == nkt - 1))
                # evict unnormalized out + Z row; free the pso bank
                hz = hozp.tile([65, SC], F32, tag="hz", name="hz")
                nc.vector.tensor_copy(hz, pso[:, :])
                eng = nc.sync if zq_sync else nc.gpsimd
                eng.dma_start(out=zqt[zrow:zrow + 1, :], in_=hz[64:65, :])
                return hz

            def zb_pair(qc, t, hoz_pair, zqt):
                """normalize head pair t of row qc (bc broadcast + muls)."""
                rz = zp.tile([2, SC], F32, tag="rz", name="rz")
                nc.vector.reciprocal_approx_fast(rz, zqt[:, :])
                rzr = zp.tile([2, SC], BF16, tag="rzr", name="rzr")
                nc.vector.tensor_copy(rzr, rz)
                bc = ps_pool.tile([128, SC], F32, tag="mm", name="bc")
                nc.tensor.matmul(
                    bc[:, :], sels[:, t * 128:(t + 1) * 128],
                    rzr[:, :], start=True, stop=True)
                for hh in range(2):
                    rows = slice(hh * 64, hh * 64 + 64)
                    nc.vector.tensor_mul(
                        ho_sb[t][rows, qc * SC:(qc + 1) * SC],
                        hoz_pair[hh][0:64, :], bc[rows, :])

            def zb_head(qc, h, hz, zqt):
                """single-head normalize (short critical path for last row)."""
                t, hh = h // 2, h % 2
                rz = zp.tile([1, SC], F32, tag="rz1", name="rz1")
                nc.vector.reciprocal_approx_fast(rz, zqt[0:1, :])
                rzr = zp.tile([1, SC], BF16, tag="rzr1", name="rzr1")
                nc.vector.tensor_copy(rzr, rz)
                bc = ps_pool.tile([64, SC], F32, tag="pso", name="bch")
                # sels[0, 0:64] is all-ones: broadcast rzr across 64 rows
                nc.tensor.matmul(
                    bc[:, :], sels[0:1, 0:64],
                    rzr[:, :], start=True, stop=True)
                nc.vector.tensor_mul(
                    ho_sb[t][hh * 64:(hh + 1) * 64, qc * SC:(qc + 1) * SC],
                    hz[0:64, :], bc[:, :])

            def oproj(qc, split_evict=False):
                for mt in range(NJT):
                    # pso-tag slots only on the last row: mid-kernel they
                    # create WAR stalls on the next row's attention
                    tag = "mm" if (mt % 2 == 0 or not split_evict) else "pso"
                    ps = ps_pool.tile([128, SC], F32, tag=tag, name="pso2")
                    for it in range(2):
                        nc.tensor.matmul(
                            ps[:, :],
                            wos[:, it, mt * 128:(mt + 1) * 128],
                            ho_sb[it][:, qc * SC:(qc + 1) * SC],
                            start=(it == 0), stop=(it == 1))
                    pe = pop.tile([128, SC], BF16, tag="pe", name="pe")
                    if split_evict and mt % 2 == 1:
                        nc.scalar.copy(pe, ps[:, :])
                        dma_eng = nc.scalar
                    else:
                        nc.vector.tensor_copy(pe, ps[:, :])
                        dma_eng = nc.sync
                    dma_eng.dma_start(
                        out=po[mt * 128:(mt + 1) * 128, qc * SC:(qc + 1) * SC],
                        in_=pe)

            # ---- schedule ----
            proj(0)
            for sc in range(NSC - 1):
                zq0 = zp.tile([2, SC], F32, tag="zq", name="zq0")
                zq1 = zp.tile([2, SC], F32, tag="zq", name="zq1")
                hoz = [att_head(sc, h, (zq0, zq1)[h // 2], zrow=h % 2)
                       for h in range(HPC)]
                proj(sc + 1)
                zb_pair(sc, 0, hoz[0:2], zq0)
                zb_pair(sc, 1, hoz[2:4], zq1)
                oproj(sc)
            # last row: per-head normalize for heads 2/3 so o-proj starts asap
            sc = NSC - 1
            zq0 = zp.tile([2, SC], F32, tag="zq", name="zq0")
            zq2 = zp.tile([1, SC], F32, tag="zq1", name="zq2")
            zq3 = zp.tile([1, SC], F32, tag="zq1", name="zq3")
            hoz = [att_head(sc, h, (zq0, zq0, zq2)[h], zrow=h % 2 if h < 2 else 0)
                   for h in range(3)]
            zb_pair(sc, 0, hoz[0:2], zq0)
            hoz.append(att_head(sc, 3, zq3, zrow=0, zq_sync=True))
            zb_head(sc, 2, hoz[2], zq2)
            zb_head(sc, 3, hoz[3], zq3)
            oproj(sc, split_evict=True)


_NC_CACHE = {}


def _get_nc(repeat: int = 1):
    if repeat not in _NC_CACHE:
        _NC_CACHE[repeat] = build(repeat)
    return _NC_CACHE[repeat]


def _host_prep(q_weight, k_weight, v_weight, o_weight, in_features):
    """Build the 8 per-core input maps."""
    ct, st = _rope_tables()
    # mask in [k, q] layout: allow k <= q
    mask01 = (np.arange(128)[:, None] <= np.arange(128)[None, :]).astype(BF16NP)
    # sel2[i, t*128+m] = 1 where i = m//64 (within-pair broadcast selector)
    sel = np.zeros((2, 256), dtype=np.float32)
    for t in range(2):
        for m in range(128):
            sel[m // 64, t * 128 + m] = 1.0
    sel = sel.astype(BF16NP)

    qw = q_weight.reshape(H, DH, D)
    kw = k_weight.reshape(H, DH, D)
    vw = v_weight.reshape(H, DH, D)

    def part_major(w_dm):  # [D, M] -> [128, NJT, M] partition-major
        return np.ascontiguousarray(
            w_dm.reshape(NJT, 128, w_dm.shape[1]).transpose(1, 0, 2))

    in_maps = []
    for c in range(NCORES):
        b, g = c // 4, c % 4
        heads = list(range(4 * g, 4 * g + 4))
        wq_c = part_major(np.ascontiguousarray(
            (0.125 * qw[heads][:, PERM64, :]).reshape(DHC, D).T).astype(BF16NP))
        wk_c = part_major(np.ascontiguousarray(
            kw[heads][:, PERM64, :].reshape(DHC, D).T).astype(BF16NP))
        wv_c = part_major(np.ascontiguousarray(
            vw[heads].reshape(DHC, D).T).astype(BF16NP))
        wo_flat = np.ascontiguousarray(
            o_weight[:, 4 * g * DH:(4 * g + 4) * DH].T).astype(BF16NP)  # [DHC, D]
        wo_c = np.ascontiguousarray(
            wo_flat.reshape(2, 128, D).transpose(1, 0, 2))
        xT_c = np.ascontiguousarray(in_features[b].T).astype(BF16NP)
        in_maps.append({
            "xT": xT_c, "wq": wq_c, "wk": wk_c, "wv": wv_c, "wo": wo_c,
            "ct": ct, "st": st, "msk": mask01,
            "sel": sel,
        })
    return in_maps


def kernel(q_weight, k_weight, v_weight, o_weight, in_features):
    q_weight = np.asarray(q_weight, dtype=np.float32)
    k_weight = np.asarray(k_weight, dtype=np.float32)
    v_weight = np.asarray(v_weight, dtype=np.float32)
    o_weight = np.asarray(o_weight, dtype=np.float32)
    in_features = np.asarray(in_features, dtype=np.float32)

    nc = _get_nc(1)
    in_maps = _host_prep(q_weight, k_weight, v_weight, o_weight, in_features)
    res = run_bass_kernel_spmd(nc, in_maps, core_ids=list(range(NCORES)))

    out = np.empty((B, S, D), dtype=np.float32)
    for b in range(B):
        acc = res.results[4 * b]["po"].astype(np.float32)
        for g in range(1, 4):
            acc += res.results[4 * b + g]["po"].astype(np.float32)
        out[b] = acc.T
    return out
